# revision 1
# baseline (speedup 1.0000x reference)
"""Trainium2 Bass kernel for the gnn_message_passing DepthWise block.

Computation (see problem reference):
    h   = x @ W1 + b1                      # [N, G]
    h   = LayerNorm(h) * ln_g + ln_b       # over channels, eps=1e-6
    acc = sum_k h[idx[:, k]] * dw_w[k]     # depthwise gather conv, K=27
    h2  = (acc + dw_b) @ W2 + b2           # [N, C_OUT]
    g   = gelu(h2)                          # exact erf form
    GRN + residual:
        Gx = ||g||_2 over rows per channel; Nx = Gx / (mean(Gx) + eps)
        out = grn_g * (g * Nx) + grn_b + g + x

Strategy: shard rows over 8 cores; each core computes h_ln for its shard
(bf16), AllGather the full [N, G] table, then each core does the neighbor
gather (one indirect DMA per 128-row tile, 27*128 rows of 512B each),
the depthwise multiply on DVE, the k-sum as 27 PSUM-accumulating identity
matmuls on PE, W2 in transposed layout, GELU on ACT, and a tiny AllReduce
for the GRN statistics.
"""

import numpy as np

from concourse import bacc, bass, mybir, tile
from concourse.bass_utils import run_bass_kernel_spmd

# ---------------------------------------------------------------- geometry
P = 128
N_CORES = 8
N = 500000
C_IN = 128
G = 256
C_OUT = 128
K = 27
FD = K * G  # gather tile free dim
EPS_LN = 1e-6
EPS_GRN = 1e-6

BF16 = mybir.dt.bfloat16
F32 = mybir.dt.float32
I32 = mybir.dt.int32
NP_BF16 = mybir.dt.np(BF16)

ADD = mybir.AluOpType.add
SUB = mybir.AluOpType.subtract
MULT = mybir.AluOpType.mult
BYPASS = mybir.AluOpType.bypass
AF = mybir.ActivationFunctionType


def cfg_tiles(n_pad):
    rpc = n_pad // N_CORES
    assert rpc % P == 0
    return rpc, rpc // P


def pad_size(n):
    q = N_CORES * P
    return (n + q - 1) // q * q


N_PAD = pad_size(N)          # 500736
RPC, T = cfg_tiles(N_PAD)    # 62592 rows/core, 489 tiles/core


# ---------------------------------------------------------------- program
def build_nc(n_pad=N_PAD, n_cores=N_CORES, gelu_func=None, debug_taps=False):
    # gelu_func override exists because CoreSim doesn't implement the Gelu
    # LUT; tests pass AF.Identity there and mirror it in the expected value.
    gelu_func = AF.Gelu if gelu_func is None else gelu_func
    rpc, n_tiles = cfg_tiles(n_pad)
    rg = [list(range(n_cores))]

    nc = bacc.Bacc(
        "TRN2", target_bir_lowering=False, debug=False, num_devices=n_cores
    )

    # ---- per-core inputs
    xbf = nc.dram_tensor("xbf", [rpc, C_IN], BF16, kind="ExternalInput")
    xrbT = nc.dram_tensor("xrbT", [C_OUT, rpc], F32, kind="ExternalInput")
    idx = nc.dram_tensor("idx", [rpc, K], I32, kind="ExternalInput")
    maskbt = nc.dram_tensor("maskbt", [P, rpc], BF16, kind="ExternalInput")
    # ---- replicated weights / constants
    w1 = nc.dram_tensor("w1", [C_IN, G], BF16, kind="ExternalInput")
    b1 = nc.dram_tensor("b1", [1, G], BF16, kind="ExternalInput")
    lngb = nc.dram_tensor("lngb", [P, G], F32, kind="ExternalInput")
    lnbb = nc.dram_tensor("lnbb", [P, G], F32, kind="ExternalInput")
    wb = nc.dram_tensor("wb", [P, FD], BF16, kind="ExternalInput")
    w2 = nc.dram_tensor("w2", [G, C_OUT], BF16, kind="ExternalInput")
    b2p = nc.dram_tensor("b2p", [C_OUT, 1], F32, kind="ExternalInput")
    grngc = nc.dram_tensor("grngc", [C_OUT, 1], F32, kind="ExternalInput")
    identb = nc.dram_tensor("identb", [P, P], BF16, kind="ExternalInput")
    onesb = nc.dram_tensor("onesb", [1, P], BF16, kind="ExternalInput")
    onescf = nc.dram_tensor("onescf", [P, 1], F32, kind="ExternalInput")
    onesrf = nc.dram_tensor("onesrf", [1, P], F32, kind="ExternalInput")
    epsc = nc.dram_tensor("epsc", [P, 2], F32, kind="ExternalInput")
    # ---- internal DRAM
    hsh = nc.dram_tensor("hsh", [rpc, G], BF16)
    table = nc.dram_tensor("table", [n_pad, G], BF16, addr_space="Shared")
    gel = nc.dram_tensor("gel", [C_OUT, rpc], BF16)
    psq_in = nc.dram_tensor("psq_in", [C_OUT, 1], F32)
    psq_out = nc.dram_tensor("psq_out", [C_OUT, 1], F32, addr_space="Shared")
    # ---- output (transposed layout; host transposes back)
    outT = nc.dram_tensor("outT", [C_OUT, rpc], F32, kind="ExternalOutput")
    if debug_taps:
        hsh_out = nc.dram_tensor("hsh_out", [rpc, G], BF16, kind="ExternalOutput")
        gel_out = nc.dram_tensor("gel_out", [C_OUT, rpc], BF16, kind="ExternalOutput")
        tbl_out = nc.dram_tensor("tbl_out", [P, G], BF16, kind="ExternalOutput")
        g0_out = nc.dram_tensor("g0_out", [P, FD], BF16, kind="ExternalOutput")
        gm0_out = nc.dram_tensor("gm0_out", [P, FD], BF16, kind="ExternalOutput")
        acc0_out = nc.dram_tensor("acc0_out", [P, G], BF16, kind="ExternalOutput")
        accT0_out = nc.dram_tensor("accT0_out", [P, 2, P], BF16, kind="ExternalOutput")

    with tile.TileContext(nc) as tc:
        with (
            tc.tile_pool(name="const", bufs=1) as cp,
            tc.tile_pool(name="work", bufs=3) as wp,
            tc.tile_pool(name="gat", bufs=3) as gp,
            tc.tile_pool(name="psum", bufs=2, space="PSUM") as pp,
        ):
            # ---------------- load constants into SBUF
            def cload(dram, shape, dtype, tag):
                t = cp.tile(shape, dtype, tag=tag)
                nc.sync.dma_start(out=t[:], in_=dram[:])
                return t

            w1_s = cload(w1, [C_IN, G], BF16, "w1")
            b1_s = cload(b1, [1, G], BF16, "b1")
            lngb_s = cload(lngb, [P, G], F32, "lngb")
            lnbb_s = cload(lnbb, [P, G], F32, "lnbb")
            wb_s = cload(wb, [P, FD], BF16, "wb")
            w2_a = cp.tile([P, C_OUT], BF16, tag="w2a")
            nc.sync.dma_start(out=w2_a[:], in_=w2[0:P, :])
            w2_b = cp.tile([P, C_OUT], BF16, tag="w2b")
            nc.sync.dma_start(out=w2_b[:], in_=w2[P:G, :])
            b2p_s = cload(b2p, [C_OUT, 1], F32, "b2p")
            grngc_s = cload(grngc, [C_OUT, 1], F32, "grngc")
            ident_s = cload(identb, [P, P], BF16, "identb")
            ones_s = cload(onesb, [1, P], BF16, "onesb")
            onescf_s = cload(onescf, [P, 1], F32, "onescf")
            onesrf_s = cload(onesrf, [1, P], F32, "onesrf")
            epsc_s = cload(epsc, [P, 2], F32, "epsc")
            # persistent per-tile GRN sumsq partials
            psq_all = cp.tile([C_OUT, n_tiles], F32, tag="psqall")

            # ---------------- phase 1: h_ln for own shard
            for t in range(n_tiles):
                r0 = t * P
                xT = wp.tile([C_IN, P], BF16, tag="xT")
                nc.sync.dma_start_transpose(
                    out=xT[:], in_=xbf[r0 : r0 + P, :]
                )
                hp = pp.tile([P, G], F32, tag="hp")
                nc.tensor.matmul(
                    out=hp[:], lhsT=ones_s[:], rhs=b1_s[:],
                    start=True, stop=False, skip_group_check=True,
                )
                nc.tensor.matmul(
                    out=hp[:], lhsT=xT[:], rhs=w1_s[:],
                    start=False, stop=True, skip_group_check=True,
                )
                stats6 = wp.tile([P, 6], F32, tag="stats6")
                nc.vector.bn_stats(out=stats6[:], in_=hp[:])
                stats2 = wp.tile([P, 2], F32, tag="stats2")
                nc.vector.bn_aggr(out=stats2[:], in_=stats6[:])
                sd = wp.tile([P, 1], F32, tag="sd")
                nc.scalar.activation(
                    out=sd[:], in_=stats2[:, 1:2], func=AF.Sqrt,
                    bias=epsc_s[:, 0:1]
                )
                rstd = wp.tile([P, 1], F32, tag="rstd")
                nc.vector.reciprocal(out=rstd[:], in_=sd[:])
                hc = wp.tile([P, G], F32, tag="hc")
                nc.vector.scalar_tensor_tensor(
                    out=hc[:], in0=hp[:], scalar=stats2[:, 0:1],
                    in1=lngb_s[:], op0=SUB, op1=MULT,
                )
                hln = wp.tile([P, G], BF16, tag="hln")
                nc.vector.scalar_tensor_tensor(
                    out=hln[:], in0=hc[:], scalar=rstd[:],
                    in1=lnbb_s[:], op0=MULT, op1=ADD,
                )
                nc.sync.dma_start(out=hsh[r0 : r0 + P, :], in_=hln[:])

            # ---------------- all-gather the feature table
            nc.gpsimd.collective_compute(
                "AllGather",
                BYPASS,
                replica_groups=rg,
                ins=[hsh.ap().opt()],
                outs=[table.ap().opt()],
            )

            # ---------------- phase 3: gather + depthwise + W2 + gelu
            for t in range(n_tiles):
                r0 = t * P
                idx_s = wp.tile([P, K], I32, tag="idx")
                nc.sync.dma_start(out=idx_s[:], in_=idx[r0 : r0 + P, :])
                g_t = gp.tile([P, FD], BF16, tag="g")
                # One indirect DMA per tap k: the only offset-AP form the HW
                # DGE implements reliably is one offset per partition with a
                # contiguous per-partition block ([P,1] offsets, [P,D] dest).
                g3v = g_t[:].rearrange("p (k c) -> p k c", k=K)
                for k in range(K):
                    nc.gpsimd.indirect_dma_start(
                        out=g3v[:, k, :],
                        out_offset=None,
                        in_=table[:, :],
                        in_offset=bass.IndirectOffsetOnAxis(
                            ap=idx_s[:, k : k + 1], axis=0
                        ),
                    )
                if debug_taps and t == 0:
                    nc.sync.dma_start(out=g0_out[:, :], in_=g_t[:])
                # depthwise multiply (in place)
                nc.vector.tensor_tensor(
                    out=g_t[:], in0=g_t[:], in1=wb_s[:], op=MULT
                )
                if debug_taps and t == 0:
                    nc.sync.dma_start(out=gm0_out[:, :], in_=g_t[:])
                # k-sum via accumulating identity matmuls
                acc = pp.tile([P, G], F32, tag="acc")
                g3 = g_t[:].rearrange("p (k g) -> p k g", k=K)
                for k in range(K):
                    nc.tensor.matmul(
                        out=acc[:], lhsT=ident_s[:], rhs=g3[:, k, :],
                        start=(k == 0), stop=(k == K - 1),
                    )
                acc_sb = wp.tile([P, G], BF16, tag="accsb")
                nc.scalar.copy(out=acc_sb[:], in_=acc[:])
                if debug_taps and t == 0:
                    nc.sync.dma_start(out=acc0_out[:, :], in_=acc_sb[:])
                # transpose acc -> [G, P] in two 128-blocks
                accT = pp.tile([P, 2, P], BF16, tag="accT")
                nc.tensor.transpose(
                    out=accT[:, 0, :], in_=acc_sb[:, 0:P], identity=ident_s[:]
                )
                nc.tensor.transpose(
                    out=accT[:, 1, :], in_=acc_sb[:, P:G], identity=ident_s[:]
                )
                accT_sb = wp.tile([P, 2, P], BF16, tag="accTsb")
                nc.scalar.copy(out=accT_sb[:, 0, :], in_=accT[:, 0, :])
                nc.scalar.copy(out=accT_sb[:, 1, :], in_=accT[:, 1, :])
                if debug_taps and t == 0:
                    nc.sync.dma_start(out=accT0_out[:, :, :], in_=accT_sb[:])
                # W2 in transposed layout: out2T[o, r]
                o2 = pp.tile([C_OUT, P], F32, tag="o2", bufs=1)
                nc.tensor.matmul(
                    out=o2[:], lhsT=w2_a[:], rhs=accT_sb[:, 0, :],
                    start=True, stop=False,
                )
                nc.tensor.matmul(
                    out=o2[:], lhsT=w2_b[:], rhs=accT_sb[:, 1, :],
                    start=False, stop=True,
                )
                gt = wp.tile([C_OUT, P], BF16, tag="gt")
                nc.scalar.activation(
                    out=gt[:], in_=o2[:], func=gelu_func, bias=b2p_s[:]
                )
                mk = wp.tile([P, P], BF16, tag="mk")
                nc.sync.dma_start(out=mk[:], in_=maskbt[:, r0 : r0 + P])
                gm = wp.tile([C_OUT, P], BF16, tag="gm")
                nc.vector.tensor_tensor(out=gm[:], in0=gt[:], in1=mk[:], op=MULT)
                sq = wp.tile([C_OUT, P], BF16, tag="sq")
                nc.scalar.activation(
                    out=sq[:], in_=gm[:], func=AF.Square,
                    accum_out=psq_all[:, t : t + 1],
                )
                nc.sync.dma_start(out=gel[:, r0 : r0 + P], in_=gm[:])

            if debug_taps:
                nc.sync.dma_start(out=hsh_out[:, :], in_=hsh[:, :])
                nc.sync.dma_start(out=gel_out[:, :], in_=gel[:, :])
                nc.sync.dma_start(out=tbl_out[:, :], in_=table[1000 : 1000 + P, :])

            # ---------------- GRN stats: reduce + all-reduce + scale
            psq_col = wp.tile([C_OUT, 1], F32, tag="psqcol")
            nc.vector.tensor_reduce(
                out=psq_col[:], in_=psq_all[:], axis=mybir.AxisListType.X, op=ADD
            )
            nc.sync.dma_start(out=psq_in[:, :], in_=psq_col[:])
            nc.gpsimd.collective_compute(
                "AllReduce",
                ADD,
                replica_groups=rg,
                ins=[psq_in.ap().opt()],
                outs=[psq_out.ap().opt()],
            )
            ssq = wp.tile([C_OUT, 1], F32, tag="ssq")
            nc.sync.dma_start(out=ssq[:], in_=psq_out[:, :])
            gx = wp.tile([C_OUT, 1], F32, tag="gx")
            nc.scalar.activation(out=gx[:], in_=ssq[:], func=AF.Sqrt, bias=0.0)
            # mean over channels via ones matmul -> [1, 1]
            smean = pp.tile([1, 1], F32, tag="small", bufs=1, name="smean")
            nc.tensor.matmul(
                out=smean[:], lhsT=onescf_s[:], rhs=gx[:], start=True, stop=True
            )
            s0 = wp.tile([1, 1], F32, tag="s0")
            # s0 = sum/C + eps  (scale during ACT copy)
            nc.scalar.activation(
                out=s0[:], in_=smean[:], func=AF.Identity,
                bias=epsc_s[0:1, 1:2], scale=1.0 / C_OUT,
            )
            rec = wp.tile([1, 1], F32, tag="rec")
            nc.vector.reciprocal(out=rec[:], in_=s0[:])
            recb = pp.tile([C_OUT, 1], F32, tag="small", bufs=1, name="recb")
            nc.tensor.matmul(
                out=recb[:], lhsT=onesrf_s[:], rhs=rec[:], start=True, stop=True
            )
            nx = wp.tile([C_OUT, 1], F32, tag="nx")
            nc.vector.tensor_tensor(out=nx[:], in0=recb[:], in1=gx[:], op=MULT)
            ga = wp.tile([C_OUT, 1], F32, tag="ga")
            nc.vector.tensor_tensor(out=ga[:], in0=nx[:], in1=grngc_s[:], op=MULT)
            a2 = wp.tile([C_OUT, 1], F32, tag="a2")
            nc.scalar.activation(out=a2[:], in_=ga[:], func=AF.Identity, bias=1.0)

            # ---------------- final: out = a2 (.) gelu + (x + grn_b)
            for t in range(n_tiles):
                r0 = t * P
                gt2 = wp.tile([C_OUT, P], BF16, tag="gt2")
                nc.sync.dma_start(out=gt2[:], in_=gel[:, r0 : r0 + P])
                xt = wp.tile([C_OUT, P], F32, tag="xt")
                nc.sync.dma_start(out=xt[:], in_=xrbT[:, r0 : r0 + P])
                u = wp.tile([C_OUT, P], F32, tag="u")
                nc.scalar.mul(out=u[:], in_=gt2[:], mul=a2[:])
                ot = wp.tile([C_OUT, P], F32, tag="ot")
                nc.vector.tensor_tensor(out=ot[:], in0=u[:], in1=xt[:], op=ADD)
                nc.sync.dma_start(out=outT[:, r0 : r0 + P], in_=ot[:])

    nc.compile()
    return nc


# ---------------------------------------------------------------- host side
def _prep_inputs(x, neighbor_idx, W1, b1, ln_g, ln_b, dw_w, dw_b, W2, b2,
                 grn_g, grn_b, n_pad=N_PAD, n_cores=N_CORES):
    rpc, n_tiles = cfg_tiles(n_pad)
    n = x.shape[0]

    xp = np.zeros((n_pad, C_IN), np.float32)
    xp[:n] = x
    idxp = np.zeros((n_pad, K), np.int32)
    idxp[:n] = neighbor_idx
    mask = np.zeros((n_pad,), np.float32)
    mask[:n] = 1.0

    xbf = xp.astype(NP_BF16)
    xrb = xp + grn_b.reshape(1, C_OUT).astype(np.float32)

    w1b = W1.astype(NP_BF16)
    b1b = b1.reshape(1, G).astype(NP_BF16)
    lngb = np.broadcast_to(ln_g.reshape(1, G), (P, G)).astype(np.float32).copy()
    lnbb = np.broadcast_to(ln_b.reshape(1, G), (P, G)).astype(np.float32).copy()
    wbf = np.broadcast_to(
        dw_w.reshape(1, FD), (P, FD)
    ).astype(NP_BF16).copy()
    w2b = W2.astype(NP_BF16)
    b2p = (dw_b.astype(np.float64) @ W2.astype(np.float64)
           + b2.astype(np.float64)).astype(np.float32).reshape(C_OUT, 1)
    grngc = grn_g.reshape(C_OUT, 1).astype(np.float32)
    identb = np.eye(P, dtype=NP_BF16)
    onesb = np.ones((1, P), NP_BF16)
    onescf = np.ones((P, 1), np.float32)
    onesrf = np.ones((1, P), np.float32)
    epsc_arr = np.broadcast_to(
        np.array([[EPS_LN, EPS_GRN]], np.float32), (P, 2)
    ).copy()

    in_maps = []
    for c in range(n_cores):
        r0 = c * rpc
        sl = slice(r0, r0 + rpc)
        mrow = mask[sl].astype(NP_BF16)
        in_maps.append({
            "xbf": np.ascontiguousarray(xbf[sl]),
            "xrbT": np.ascontiguousarray(xrb[sl].T),
            "idx": np.ascontiguousarray(idxp[sl]),
            "maskbt": np.ascontiguousarray(
                np.broadcast_to(mrow.reshape(1, rpc), (P, rpc))
            ),
            "w1": w1b, "b1": b1b, "lngb": lngb, "lnbb": lnbb,
            "wb": wbf, "w2": w2b, "b2p": b2p, "grngc": grngc,
            "identb": identb, "onesb": onesb,
            "onescf": onescf, "onesrf": onesrf, "epsc": epsc_arr,
        })
    return in_maps


_NC_CACHE = {}


def _get_nc(n_pad=N_PAD, n_cores=N_CORES):
    key = (n_pad, n_cores)
    if key not in _NC_CACHE:
        _NC_CACHE[key] = build_nc(n_pad, n_cores)
    return _NC_CACHE[key]


def kernel(x, neighbor_idx, W1, b1, ln_g, ln_b, dw_w, dw_b, W2, b2,
           grn_g, grn_b, _trace=False, _trace_cores=None):
    x = np.asarray(x, np.float32)
    neighbor_idx = np.asarray(neighbor_idx, np.int32)
    args = [np.asarray(a) for a in
            (W1, b1, ln_g, ln_b, dw_w, dw_b, W2, b2, grn_g, grn_b)]

    nc = _get_nc()
    in_maps = _prep_inputs(x, neighbor_idx, *args)
    res = run_bass_kernel_spmd(
        nc, in_maps, core_ids=list(range(N_CORES)),
        trace=_trace, trace_cores=_trace_cores,
    )
    n = x.shape[0]
    rpc, _ = cfg_tiles(N_PAD)
    out = np.empty((N_PAD, C_OUT), np.float32)
    for c in range(N_CORES):
        out[c * rpc : (c + 1) * rpc] = res.results[c]["outT"].T
    if _trace:
        kernel._last_result = res
    return out[:n]



# revision 7
# speedup vs baseline: 1.1151x; 1.1151x over previous
"""Trainium2 Bass kernel for the gnn_message_passing DepthWise block.

Computation (see problem reference):
    h   = x @ W1 + b1                      # [N, G]
    h   = LayerNorm(h) * ln_g + ln_b       # over channels, eps=1e-6
    acc = sum_k h[idx[:, k]] * dw_w[k]     # depthwise gather conv, K=27
    h2  = (acc + dw_b) @ W2 + b2           # [N, C_OUT]
    g   = gelu(h2)                          # exact erf form
    GRN + residual:
        Gx = ||g||_2 over rows per channel; Nx = Gx / (mean(Gx) + eps)
        out = grn_g * (g * Nx) + grn_b + g + x

Strategy: shard rows over 8 cores.  LayerNorm affine is folded into the
depthwise weights on the host (table stores the pure normalized value
(h-mu)*rstd; dw_w' = dw_w*ln_g; the ln_b term becomes a constant bias
folded into b2).  Each core computes its normalized shard, the shards are
AllGathered in chunks (overlapped with phase-1 compute), then each core
gathers neighbors with 27 [P,1]-offset indirect DMAs per 128-row tile
(the only offset form the HW DGE implements), multiplies by the folded
depthwise weights on DVE, k-sums with PSUM-accumulating identity matmuls,
projects through W2, applies GELU, and accumulates GRN statistics.  A tiny
AllReduce produces the GRN scale; a final batched pass applies it plus the
residual in bf16.
"""

import numpy as np

from concourse import bacc, bass, mybir, tile
from concourse.bass_utils import run_bass_kernel_spmd

# ---------------------------------------------------------------- geometry
P = 128
N_CORES = 8
N = 500000
C_IN = 128
G = 256
C_OUT = 128
K = 27
FD = K * G
EPS_LN = 1e-6
EPS_GRN = 1e-6

B1 = 4   # phase-1 slab batch (tiles)
B3 = 8   # idx slab batch
BF = 4   # final slab batch

BF16 = mybir.dt.bfloat16
F32 = mybir.dt.float32
I32 = mybir.dt.int32
NP_BF16 = mybir.dt.np(BF16)

ADD = mybir.AluOpType.add
SUB = mybir.AluOpType.subtract
MULT = mybir.AluOpType.mult
BYPASS = mybir.AluOpType.bypass
AF = mybir.ActivationFunctionType


def cfg_tiles(n_pad):
    rpc = n_pad // N_CORES
    assert rpc % P == 0
    return rpc, rpc // P


def pad_size(n):
    q = N_CORES * P
    return (n + q - 1) // q * q


N_PAD = pad_size(N)          # 500736
RPC, T = cfg_tiles(N_PAD)    # 62592 rows/core, 489 tiles/core

# AllGather chunk boundaries, in tiles (multiples of B1=4 so each chunk
# completes exactly at a phase-1 slab boundary).  The table is stored
# chunk-major ([chunk, core, local_row, G] flattened) so every chunk's
# AllGather output is contiguous; neighbor indices are remapped on the
# host to match.
AG_CHUNKS = [0, 124, 248, 372, 489]


# ---------------------------------------------------------------- program
def build_nc(n_pad=N_PAD, n_cores=N_CORES, gelu_func=None):
    gelu_func = AF.Gelu if gelu_func is None else gelu_func
    rpc, n_tiles = cfg_tiles(n_pad)
    rg = [list(range(n_cores))]

    nc = bacc.Bacc(
        "TRN2", target_bir_lowering=False, debug=False, num_devices=n_cores
    )

    # ---- per-core inputs
    xTd = nc.dram_tensor("xTd", [C_IN, rpc], BF16, kind="ExternalInput")
    xrbT = nc.dram_tensor("xrbT", [C_OUT, rpc], BF16, kind="ExternalInput")
    idx = nc.dram_tensor("idx", [rpc, K], I32, kind="ExternalInput")
    # ---- replicated weights / constants
    w1 = nc.dram_tensor("w1", [C_IN, G], BF16, kind="ExternalInput")
    b1 = nc.dram_tensor("b1", [1, G], BF16, kind="ExternalInput")
    wb = nc.dram_tensor("wb", [P, FD], BF16, kind="ExternalInput")
    w2 = nc.dram_tensor("w2", [G, C_OUT], BF16, kind="ExternalInput")
    b2p = nc.dram_tensor("b2p", [C_OUT, 1], F32, kind="ExternalInput")
    grngc = nc.dram_tensor("grngc", [C_OUT, 1], F32, kind="ExternalInput")
    identb = nc.dram_tensor("identb", [P, P], BF16, kind="ExternalInput")
    onesb = nc.dram_tensor("onesb", [1, P], BF16, kind="ExternalInput")
    onescf = nc.dram_tensor("onescf", [P, 1], F32, kind="ExternalInput")
    onesrf = nc.dram_tensor("onesrf", [1, P], F32, kind="ExternalInput")
    epsc = nc.dram_tensor("epsc", [P, 2], F32, kind="ExternalInput")
    # ---- internal DRAM
    hsh = nc.dram_tensor("hsh", [rpc, G], BF16)
    table = nc.dram_tensor("table", [n_pad, G], BF16, addr_space="Shared")
    gel = nc.dram_tensor("gel", [C_OUT, rpc], BF16)
    psq_in = nc.dram_tensor("psq_in", [C_OUT, 1], F32)
    psq_out = nc.dram_tensor("psq_out", [C_OUT, 1], F32, addr_space="Shared")
    # ---- output (transposed bf16; host transposes + casts back)
    outT = nc.dram_tensor("outT", [C_OUT, rpc], BF16, kind="ExternalOutput")



    with tile.TileContext(nc) as tc:
        with (
            tc.tile_pool(name="const", bufs=1) as cp,
            tc.tile_pool(name="ph1", bufs=6) as p1,
            tc.tile_pool(name="work", bufs=4) as wp,
            tc.tile_pool(name="gat", bufs=3) as gp,
            tc.tile_pool(name="psum", bufs=2, space="PSUM") as pp,
        ):
            # ---------------- load constants into SBUF
            def cload(dram, shape, dtype, tag):
                t = cp.tile(shape, dtype, tag=tag)
                nc.sync.dma_start(out=t[:], in_=dram[:])
                return t

            w1_s = cload(w1, [C_IN, G], BF16, "w1")
            b1_s = cload(b1, [1, G], BF16, "b1")
            wb_s = cload(wb, [P, FD], BF16, "wb")
            w2_a = cp.tile([P, C_OUT], BF16, tag="w2a")
            nc.sync.dma_start(out=w2_a[:], in_=w2[0:P, :])
            w2_b = cp.tile([P, C_OUT], BF16, tag="w2b")
            nc.sync.dma_start(out=w2_b[:], in_=w2[P:G, :])
            b2p_s = cload(b2p, [C_OUT, 1], F32, "b2p")
            grngc_s = cload(grngc, [C_OUT, 1], F32, "grngc")
            ident_s = cload(identb, [P, P], BF16, "identb")
            ones_s = cload(onesb, [1, P], BF16, "onesb")
            onescf_s = cload(onescf, [P, 1], F32, "onescf")
            onesrf_s = cload(onesrf, [1, P], F32, "onesrf")
            epsc_s = cload(epsc, [P, 2], F32, "epsc")
            psq_all = cp.tile([C_OUT, n_tiles], F32, tag="psqall")

            # ---------------- phase 1: normalized table rows for own shard
            hsh_v = hsh.ap().rearrange("(b p) g -> p b g", p=P)
            n_slabs = n_tiles // B1
            assert n_slabs * B1 == n_tiles + 0 or True
            slab_starts = list(range(0, n_tiles, B1))
            ag_next = 1

            for s0 in slab_starts:
                nb = min(B1, n_tiles - s0)
                r0 = s0 * P
                xs = p1.tile([C_IN, B1 * P], BF16, tag="xs")
                nc.sync.dma_start(
                    out=xs[:, : nb * P], in_=xTd[:, r0 : r0 + nb * P]
                )
                hs = p1.tile([P, B1, G], BF16, tag="hs")
                for j in range(nb):
                    hp = pp.tile([P, G], F32, tag="hp")
                    nc.tensor.matmul(
                        out=hp[:], lhsT=ones_s[:], rhs=b1_s[:],
                        start=True, stop=False, skip_group_check=True,
                    )
                    nc.tensor.matmul(
                        out=hp[:], lhsT=xs[:, j * P : (j + 1) * P], rhs=w1_s[:],
                        start=False, stop=True, skip_group_check=True,
                    )
                    stats6 = p1.tile([P, 6], F32, tag="stats6")
                    nc.vector.bn_stats(out=stats6[:], in_=hp[:])
                    stats2 = p1.tile([P, 2], F32, tag="stats2")
                    nc.vector.bn_aggr(out=stats2[:], in_=stats6[:])
                    sd = p1.tile([P, 1], F32, tag="sd")
                    nc.scalar.activation(
                        out=sd[:], in_=stats2[:, 1:2], func=AF.Sqrt,
                        bias=epsc_s[:, 0:1],
                    )
                    rstd = p1.tile([P, 1], F32, tag="rstd")
                    nc.vector.reciprocal(out=rstd[:], in_=sd[:])
                    # nmr = (mu * -1) * rstd
                    nmr = p1.tile([P, 1], F32, tag="nmr")
                    nc.vector.scalar_tensor_tensor(
                        out=nmr[:], in0=stats2[:, 0:1], scalar=-1.0,
                        in1=rstd[:], op0=MULT, op1=MULT,
                    )
                    # normalized row in one ACT pass: (hp - mu) * rstd
                    nc.scalar.activation(
                        out=hs[:, j, :], in_=hp[:], func=AF.Identity,
                        scale=rstd[:], bias=nmr[:],
                    )
                nc.sync.dma_start(
                    out=hsh_v[:, s0 : s0 + nb, :], in_=hs[:, :nb, :]
                )
                # fire AllGather chunks as soon as their tiles are written;
                # chunk-major table layout keeps each output contiguous
                while ag_next < len(AG_CHUNKS) and s0 + nb >= AG_CHUNKS[ag_next]:
                    c0, c1 = AG_CHUNKS[ag_next - 1] * P, AG_CHUNKS[ag_next] * P
                    nc.gpsimd.collective_compute(
                        "AllGather",
                        BYPASS,
                        replica_groups=rg,
                        ins=[hsh.ap()[c0:c1, :].opt()],
                        outs=[
                            table.ap()[
                                n_cores * c0 : n_cores * c1, :
                            ].opt()
                        ],
                    )
                    ag_next += 1

            # ---------------- phase 3: gather + depthwise + W2 + gelu
            idx_v = idx.ap().rearrange("(b p) k -> p b k", p=P)
            idx_s = None
            for t in range(n_tiles):
                if t % B3 == 0:
                    nb = min(B3, n_tiles - t)
                    idx_s = wp.tile([P, B3, K], I32, tag="idx")
                    nc.sync.dma_start(
                        out=idx_s[:, :nb, :], in_=idx_v[:, t : t + nb, :]
                    )
                j3 = t % B3
                r0 = t * P
                g_t = gp.tile([P, FD], BF16, tag="g")
                g3v = g_t[:].rearrange("p (k c) -> p k c", k=K)
                for k in range(K):
                    nc.gpsimd.indirect_dma_start(
                        out=g3v[:, k, :],
                        out_offset=None,
                        in_=table[:, :],
                        in_offset=bass.IndirectOffsetOnAxis(
                            ap=idx_s[:, j3, k : k + 1], axis=0
                        ),
                    )
                # depthwise multiply (in place) with folded ln_g
                nc.vector.tensor_tensor(
                    out=g_t[:], in0=g_t[:], in1=wb_s[:], op=MULT
                )
                # k-sum via accumulating identity matmuls
                acc = pp.tile([P, G], F32, tag="acc")
                g3 = g_t[:].rearrange("p (k g) -> p k g", k=K)
                for k in range(K):
                    nc.tensor.matmul(
                        out=acc[:], lhsT=ident_s[:], rhs=g3[:, k, :],
                        start=(k == 0), stop=(k == K - 1),
                    )
                acc_sb = wp.tile([P, G], BF16, tag="accsb")
                nc.scalar.copy(out=acc_sb[:], in_=acc[:])
                accT = pp.tile([P, 2, P], BF16, tag="accT")
                nc.tensor.transpose(
                    out=accT[:, 0, :], in_=acc_sb[:, 0:P], identity=ident_s[:]
                )
                nc.tensor.transpose(
                    out=accT[:, 1, :], in_=acc_sb[:, P:G], identity=ident_s[:]
                )
                accT_sb = wp.tile([P, 2, P], BF16, tag="accTsb")
                nc.scalar.copy(out=accT_sb[:, 0, :], in_=accT[:, 0, :])
                nc.scalar.copy(out=accT_sb[:, 1, :], in_=accT[:, 1, :])
                o2 = pp.tile([C_OUT, P], F32, tag="o2", bufs=1)
                nc.tensor.matmul(
                    out=o2[:], lhsT=w2_a[:], rhs=accT_sb[:, 0, :],
                    start=True, stop=False,
                )
                nc.tensor.matmul(
                    out=o2[:], lhsT=w2_b[:], rhs=accT_sb[:, 1, :],
                    start=False, stop=True,
                )
                gt = wp.tile([C_OUT, P], BF16, tag="gt")
                nc.scalar.activation(
                    out=gt[:], in_=o2[:], func=gelu_func, bias=b2p_s[:]
                )
                sq = wp.tile([C_OUT, P], BF16, tag="sq")
                nc.scalar.activation(
                    out=sq[:], in_=gt[:], func=AF.Square,
                    accum_out=psq_all[:, t : t + 1],
                )
                nc.sync.dma_start(out=gel[:, r0 : r0 + P], in_=gt[:])

            # ---------------- GRN stats: reduce + all-reduce + scale
            psq_col = wp.tile([C_OUT, 1], F32, tag="psqcol")
            nc.vector.tensor_reduce(
                out=psq_col[:], in_=psq_all[:], axis=mybir.AxisListType.X, op=ADD
            )
            nc.sync.dma_start(out=psq_in[:, :], in_=psq_col[:])
            nc.gpsimd.collective_compute(
                "AllReduce",
                ADD,
                replica_groups=rg,
                ins=[psq_in.ap().opt()],
                outs=[psq_out.ap().opt()],
            )
            ssq = wp.tile([C_OUT, 1], F32, tag="ssq")
            nc.sync.dma_start(out=ssq[:], in_=psq_out[:, :])
            gx = wp.tile([C_OUT, 1], F32, tag="gx")
            nc.scalar.activation(out=gx[:], in_=ssq[:], func=AF.Sqrt, bias=0.0)
            smean = pp.tile([1, 1], F32, tag="small", bufs=1, name="smean")
            nc.tensor.matmul(
                out=smean[:], lhsT=onescf_s[:], rhs=gx[:], start=True, stop=True
            )
            s0t = wp.tile([1, 1], F32, tag="s0")
            nc.scalar.activation(
                out=s0t[:], in_=smean[:], func=AF.Identity,
                bias=epsc_s[0:1, 1:2], scale=1.0 / C_OUT,
            )
            rec = wp.tile([1, 1], F32, tag="rec")
            nc.vector.reciprocal(out=rec[:], in_=s0t[:])
            recb = pp.tile([C_OUT, 1], F32, tag="small", bufs=1, name="recb")
            nc.tensor.matmul(
                out=recb[:], lhsT=onesrf_s[:], rhs=rec[:], start=True, stop=True
            )
            nx = wp.tile([C_OUT, 1], F32, tag="nx")
            nc.vector.tensor_tensor(out=nx[:], in0=recb[:], in1=gx[:], op=MULT)
            ga = wp.tile([C_OUT, 1], F32, tag="ga")
            nc.vector.tensor_tensor(out=ga[:], in0=nx[:], in1=grngc_s[:], op=MULT)
            a2 = wp.tile([C_OUT, 1], F32, tag="a2")
            nc.scalar.activation(out=a2[:], in_=ga[:], func=AF.Identity, bias=1.0)

            # ---------------- final: out = a2 (.) gelu + (x + grn_b), batched
            for s0 in range(0, n_tiles, BF):
                nb = min(BF, n_tiles - s0)
                r0 = s0 * P
                w = nb * P
                gt2 = wp.tile([C_OUT, BF * P], BF16, tag="gt2")
                nc.sync.dma_start(out=gt2[:, :w], in_=gel[:, r0 : r0 + w])
                xt = wp.tile([C_OUT, BF * P], BF16, tag="xt")
                nc.sync.dma_start(out=xt[:, :w], in_=xrbT[:, r0 : r0 + w])
                u = wp.tile([C_OUT, BF * P], F32, tag="u")
                nc.scalar.mul(out=u[:, :w], in_=gt2[:, :w], mul=a2[:])
                ot = wp.tile([C_OUT, BF * P], BF16, tag="ot")
                nc.vector.tensor_tensor(
                    out=ot[:, :w], in0=u[:, :w], in1=xt[:, :w], op=ADD
                )
                nc.sync.dma_start(out=outT[:, r0 : r0 + w], in_=ot[:, :w])

    nc.compile()
    return nc


# ---------------------------------------------------------------- host side
def _prep_inputs(x, neighbor_idx, W1, b1, ln_g, ln_b, dw_w, dw_b, W2, b2,
                 grn_g, grn_b, n_pad=N_PAD, n_cores=N_CORES):
    rpc, n_tiles = cfg_tiles(n_pad)
    n = x.shape[0]

    xp = np.zeros((n_pad, C_IN), np.float32)
    xp[:n] = x
    idxp = np.zeros((n_pad, K), np.int32)
    idxp[:n] = neighbor_idx

    # remap global row index -> chunk-major table position:
    # table2[(cum[c] * n_cores + s * sz[c]) + (r - cum[c])] = h[s * rpc + r]
    bounds = np.array(AG_CHUNKS, np.int64) * P          # local-row chunk bounds
    sz = np.diff(bounds)                                # rows per chunk
    base2 = np.concatenate(([0], np.cumsum(sz * n_cores)))  # table2 chunk starts
    s_id = idxp.astype(np.int64) // rpc
    r_id = idxp.astype(np.int64) % rpc
    c_id = np.searchsorted(bounds, r_id, side="right") - 1
    idxp = (base2[c_id] + s_id * sz[c_id] + (r_id - bounds[c_id])).astype(
        np.int32
    )

    xT = np.ascontiguousarray(xp.T).astype(NP_BF16)          # [C_IN, n_pad]
    xrb = (xp + grn_b.reshape(1, C_OUT)).T.astype(NP_BF16)   # [C_OUT, n_pad]

    w1b = W1.astype(NP_BF16)
    b1b = b1.reshape(1, G).astype(NP_BF16)
    # fold LN affine: table holds (h-mu)*rstd; dw' = dw_w * ln_g;
    # ln_b contributes a constant bias ln_b * sum_k dw_w[k] per channel.
    dw_f = (dw_w.astype(np.float64) * ln_g.reshape(1, G).astype(np.float64))
    wbf = np.broadcast_to(
        dw_f.reshape(1, FD).astype(np.float32), (P, FD)
    ).astype(NP_BF16).copy()
    cbias = (ln_b.astype(np.float64)
             * dw_w.astype(np.float64).sum(axis=0))           # [G]
    w2b = W2.astype(NP_BF16)
    b2p = ((dw_b.astype(np.float64) + cbias) @ W2.astype(np.float64)
           + b2.astype(np.float64)).astype(np.float32).reshape(C_OUT, 1)
    grngc = grn_g.reshape(C_OUT, 1).astype(np.float32)
    identb = np.eye(P, dtype=NP_BF16)
    onesb = np.ones((1, P), NP_BF16)
    onescf = np.ones((P, 1), np.float32)
    onesrf = np.ones((1, P), np.float32)
    epsc_arr = np.broadcast_to(
        np.array([[EPS_LN, EPS_GRN]], np.float32), (P, 2)
    ).copy()

    in_maps = []
    for c in range(n_cores):
        r0 = c * rpc
        sl = slice(r0, r0 + rpc)
        in_maps.append({
            "xTd": np.ascontiguousarray(xT[:, sl]),
            "xrbT": np.ascontiguousarray(xrb[:, sl]),
            "idx": np.ascontiguousarray(idxp[sl]),
            "w1": w1b, "b1": b1b,
            "wb": wbf, "w2": w2b, "b2p": b2p, "grngc": grngc,
            "identb": identb, "onesb": onesb,
            "onescf": onescf, "onesrf": onesrf, "epsc": epsc_arr,
        })
    return in_maps


_NC_CACHE = {}


def _get_nc(n_pad=N_PAD, n_cores=N_CORES):
    key = (n_pad, n_cores)
    if key not in _NC_CACHE:
        _NC_CACHE[key] = build_nc(n_pad, n_cores)
    return _NC_CACHE[key]


def kernel(x, neighbor_idx, W1, b1, ln_g, ln_b, dw_w, dw_b, W2, b2,
           grn_g, grn_b, _trace=False, _trace_cores=None):
    x = np.asarray(x, np.float32)
    neighbor_idx = np.asarray(neighbor_idx, np.int32)
    args = [np.asarray(a) for a in
            (W1, b1, ln_g, ln_b, dw_w, dw_b, W2, b2, grn_g, grn_b)]

    nc = _get_nc()
    in_maps = _prep_inputs(x, neighbor_idx, *args)
    res = run_bass_kernel_spmd(
        nc, in_maps, core_ids=list(range(N_CORES)),
        trace=_trace, trace_cores=_trace_cores,
    )
    n = x.shape[0]
    rpc, _ = cfg_tiles(N_PAD)
    out = np.empty((N_PAD, C_OUT), np.float32)
    for c in range(N_CORES):
        out[c * rpc : (c + 1) * rpc] = res.results[c]["outT"].T.astype(np.float32)
    if _trace:
        kernel._last_result = res
    return out[:n]
